# revision 27
# baseline (speedup 1.0000x reference)
"""Trainium2 Bass kernel for nn_BinaryDense: out = x @ (sum_k sign(b_k)*a_k) + bias.

Shapes (hardcoded): x [4096,4096] f32, b [4,4096,4096] f32, a [4,4096] f32,
bias [4096] f32 -> out [4096,4096] f32.

Strategy: tensor-parallel over the output (units) dim across 8 NeuronCores.
Core c owns O-columns [c*512, (c+1)*512).

Per core: one bf16 matmul x @ w with w built on-chip.
  w[:, oc] = sum_k copysign(a[k,oc], b[k,:,oc]); b arrives bf16 in
  [I, K, O_c] (k-major) layout. Build per 128-row k-tile is 3 DVE ops:
    contrib = (b & 0x80008000) | a   (one fused scalar_tensor_tensor, int32)
    t = contrib[0:2] + contrib[2:4]  (bf16 add, 1024 wide)
    w = t[0] + t[1]                  (bf16 add, 512 wide)

Schedule: software-pipelined mb stream. The 32 m-tiles form 8 mbs of 4 per
k-block; PSUM holds two wide [128, 2048] f32 tiles (4 banks each), i.e. two
mbs in flight. mb j+1's kt-sweep is offset ~half a sweep from mb j's, and
mb j+2 starts E extra visit-slots after mb j ends, so mb j's psum eviction
(two half-wide [128,1024] DVE adds into an fp32 SBUF accumulator) hides
under mb j+1's solo visits instead of stalling the PE. The pipeline flows
seamlessly across k-block boundaries. Per visit (mb, kt) the PE runs 4
matmuls (512 moving cols each); start/stop flags bound each k-block's
accumulation group per psum slice.

Measured hardware notes:
- Any GPSIMD Q7 activity (partition_broadcast, gpsimd tensor ops --
  anything needing LOAD_LIB) risks chip downclock; GpSimd issues DMA
  descriptors only. (The chip also lotteries between ~2.4 and ~2.0 GHz
  run-to-run regardless of the kernel.)
- DMA queue bandwidth divides ~equally among ACTIVE queues (~400-450
  GB/s aggregate, ~110-150 GB/s per busy queue during warmup), and the
  first HWDGE byte moves only at t~8us. So the warmup-critical tiles
  (a+b0..b3 and the first xt) are spread ACROSS the sync/scalar/gpsimd
  queues to land concurrently, bias is deferred to mid-kernel, and
  kb1's b tiles self-pace via bpool back-pressure.
- kb0's psum evictions are ACT-engine copies (bias folded in at kb
  NKB-2 when the DVE has slack) so the DVE warmup is builds-only.

DMA layout: xt comes as kt-PAIR tiles [128, 1024] (halves descriptor
count; each issue costs ~0.7us of queue time) from a host
pair-interleaved copy of x^T, alternating scalar/sync queues; out
stores alternate queues likewise.

Host side only reshapes/casts/shards (no math): x^T bf16 pair-interleaved,
b -> [I,K,O] bf16, a broadcast rows, bias tiled 4x broadcast rows (bf16).
"""

import sys

if "/opt/trn_rl_repo" not in sys.path:
    sys.path.insert(0, "/opt/trn_rl_repo")

import numpy as np
import ml_dtypes

BF16 = ml_dtypes.bfloat16

B = 4096   # batch rows of x
I = 4096   # input dim (contraction)
O = 4096   # output dim (sharded)
K = 4      # binary bases
NCORES = 8
OC = O // NCORES   # 512 output cols per core
P = 128

KT = I // P        # 32 k-tiles (contraction)
MT = B // P        # 32 m-tiles (output rows)
NMB = 8            # mbs (of 4 m-tiles) per k-block sweep

SIGNMASK = -2147450880  # 0x80008000: bf16 sign-bit pair as int32


def _build_program():
    import os
    import concourse.bass as bass
    import concourse.mybir as mybir
    from concourse import bacc
    from concourse.tile import TileContext

    nc = bacc.Bacc(None, target_bir_lowering=False)

    b_re = nc.declare_dram_parameter("b_re", [I, K * OC], mybir.dt.bfloat16, isOutput=False)
    a_b = nc.declare_dram_parameter("a_b", [P, K * OC], mybir.dt.bfloat16, isOutput=False)
    # x^T, pair-interleaved: [ktp*128+p, mb*1024 + half*512 + c]
    xT4 = nc.declare_dram_parameter("xT4", [I // 2, 2 * B], mybir.dt.bfloat16, isOutput=False)
    bias_w = nc.declare_dram_parameter("bias_w", [P, 4 * OC], mybir.dt.bfloat16, isOutput=False)
    out = nc.declare_dram_parameter("out", [B, OC], mybir.dt.float32, isOutput=True)

    K_BLOCKS = [int(s) for s in os.environ.get("BK_KBLOCKS", "4,4,6,8,10").split(",")]
    assert sum(K_BLOCKS) == KT
    assert all(kb % 2 == 0 for kb in K_BLOCKS)
    NKB = len(K_BLOCKS)
    k_starts = [sum(K_BLOCKS[:i]) for i in range(NKB)]
    N_DUM = int(os.environ.get("BK_DUMMIES", "20"))
    E_SLOTS = int(os.environ.get("BK_E", "3"))
    N_FILL = int(os.environ.get("BK_FILL", "9"))
    LOOKV = int(os.environ.get("BK_LOOK", "7"))
    AHEAD = int(os.environ.get("BK_AHEAD", "40"))

    # ---- software-pipeline schedule ----
    M = [(kb, mb) for kb in range(NKB) for mb in range(NMB)]
    lens = [K_BLOCKS[kb] for kb, mb in M]
    starts = []
    for j in range(len(M)):
        s = 0 if j == 0 else starts[j - 1] + (lens[j - 1] + 1) // 2
        if j >= 2:
            s = max(s, starts[j - 2] + lens[j - 2] + E_SLOTS)
        starts.append(s)
    for j in range(2, len(M)):
        assert starts[j] >= starts[j - 2] + lens[j - 2], "psum overcommit"
    visits = sorted(
        (starts[j] + i, j, i) for j in range(len(M)) for i in range(lens[j])
    )
    NV = len(visits)
    assert K_BLOCKS[0] >= 4 and NKB >= 3
    assert NV == NMB * KT  # 256 visits = 1024 matmuls / 4

    def glob_kt(j, i):
        return k_starts[M[j][0]] + i

    first_seen = {}
    vidx = {}
    for v, (_, j, i) in enumerate(visits):
        vidx[(j, i)] = v
        kt = glob_kt(j, i)
        if kt not in first_seen:
            first_seen[kt] = v
    assert sorted(first_seen) == list(range(KT))
    assert all(first_seen[k] <= first_seen[k + 1] for k in range(KT - 1))

    # xt pair-fetches: (j, q) covers visits (j, 2q) and (j, 2q+1).
    # Emit each fetch LOOKV visits before its first use.
    fetch_at = [[] for _ in range(NV)]
    n_fetch = 0
    for j in range(len(M)):
        for q in range(lens[j] // 2):
            use = vidx[(j, 2 * q)]
            fetch_at[max(0, use - LOOKV)].append((j, q))
            n_fetch += 1

    with TileContext(nc) as tc:
        with (
            tc.tile_pool(name="const", bufs=1) as const,
            tc.tile_pool(name="bpool", bufs=6) as bpool,
            tc.tile_pool(name="cpool", bufs=3) as cpool,
            tc.tile_pool(name="tpool", bufs=3) as tpool,
            tc.tile_pool(name="wpool", bufs=1) as wpool,
            tc.tile_pool(name="xpool", bufs=8) as xpool,
            tc.tile_pool(name="apool", bufs=1) as apool,
            tc.tile_pool(name="opool", bufs=2) as opool,
            tc.tile_pool(name="psum", bufs=2, space="PSUM") as psum_pool,
        ):
            # const tiles declared here; their DMAs are emitted inside the
            # hand-ordered prime sequence below (queue order = emission order)
            a_tile = const.tile([P, K * OC], mybir.dt.bfloat16)
            bias_tile = const.tile([P, 4 * OC], mybir.dt.bfloat16)

            mask_tile = const.tile([P, 1], mybir.dt.int32)
            nc.vector.memset(mask_tile[:], SIGNMASK)
            dummy_w = const.tile([P, P], mybir.dt.bfloat16)
            nc.vector.memset(dummy_w[:], 0)
            dummy_rhs = const.tile([P, OC], mybir.dt.bfloat16)
            nc.vector.memset(dummy_rhs[:], 0)
            act_warm = const.tile([P, 8], mybir.dt.float32)
            nc.scalar.copy(out=act_warm[:], in_=dummy_rhs.bitcast(mybir.dt.float32)[:, 0:8])

            # ---- w-build machinery ----
            b_live = {}
            w_tiles = [None] * KT

            def emit_bdma(kt):
                # queue bandwidth divides ~equally among ACTIVE queues, so
                # the warmup b tiles are spread across all three queues to
                # land concurrently; the rest rides sync (idle mid-kernel).
                b_tile = bpool.tile([P, K * OC], mybir.dt.bfloat16, name="b_tile")
                if kt == 0:
                    eng = nc.scalar
                elif kt == 1 or kt >= k_starts[2]:
                    eng = nc.sync
                else:
                    eng = nc.gpsimd
                eng.dma_start(out=b_tile[:], in_=b_re[kt * P:(kt + 1) * P, :])
                b_live[kt] = b_tile

            def emit_build(kt):
                b_tile = b_live.pop(kt)
                contrib = cpool.tile([P, K * OC], mybir.dt.bfloat16, name="contrib")
                nc.vector.scalar_tensor_tensor(
                    out=contrib.bitcast(mybir.dt.int32)[:],
                    in0=b_tile.bitcast(mybir.dt.int32)[:],
                    scalar=mask_tile[:, 0:1],
                    in1=a_tile.bitcast(mybir.dt.int32)[:],
                    op0=mybir.AluOpType.bitwise_and,
                    op1=mybir.AluOpType.bitwise_or,
                )
                t_tile = tpool.tile([P, 2 * OC], mybir.dt.bfloat16, name="t_tile")
                nc.vector.tensor_tensor(
                    out=t_tile[:],
                    in0=contrib[:, 0:2 * OC],
                    in1=contrib[:, 2 * OC:4 * OC],
                    op=mybir.AluOpType.add,
                )
                w_tile = wpool.tile([P, OC], mybir.dt.bfloat16, name=f"w_{kt}")
                nc.vector.tensor_tensor(
                    out=w_tile[:],
                    in0=t_tile[:, 0:OC],
                    in1=t_tile[:, OC:2 * OC],
                    op=mybir.AluOpType.add,
                )
                w_tiles[kt] = w_tile

            xt_tiles = {}

            def emit_xt_pair(j, q):
                if (j, q) in xt_tiles:
                    return
                kb, mb = M[j]
                ktp = (k_starts[kb] + 2 * q) // 2
                xt = xpool.tile([P, 8 * P], mybir.dt.bfloat16, name="xt")
                xeng = nc.scalar if (j + q) % 2 == 0 else nc.sync
                xeng.dma_start(
                    out=xt[:],
                    in_=xT4[ktp * P:(ktp + 1) * P, mb * 8 * P:(mb + 1) * 8 * P],
                )
                xt_tiles[(j, q)] = xt

            # ---- prime: hand-ordered critical chain on the sync queue:
            # b0, a, X00, b1, X01, b2, X10, b3, X11, X20. Bias (bf16) heads
            # the gpsimd queue, followed later by kb1's b tiles. ----
            n_prime = 0
            while n_prime < KT and first_seen[n_prime] < AHEAD:
                n_prime += 1
            nc.sync.dma_start(out=a_tile[:], in_=a_b[:, :])
            emit_bdma(0)          # scalar head
            emit_bdma(1)          # sync (behind a)
            emit_bdma(3)          # gpsimd head
            emit_bdma(2)          # gpsimd
            emit_xt_pair(0, 0)    # scalar (behind b0)
            emit_build(0)
            emit_build(1)
            emit_xt_pair(0, 1)
            emit_build(2)
            emit_xt_pair(1, 0)
            emit_build(3)
            emit_xt_pair(1, 1)
            emit_xt_pair(2, 0)
            build_cursor = 4
            while build_cursor < n_prime:
                emit_bdma(build_cursor)
                emit_build(build_cursor)
                build_cursor += 1

            for _ in range(N_DUM):
                dps = psum_pool.tile([P, 4 * OC], mybir.dt.float32, name="psw")
                nc.tensor.matmul(dps[:, 0:OC], dummy_w[:], dummy_rhs[:],
                                 start=True, stop=True)

            # ---- main loop over scheduled visits ----
            ps_tiles = {}
            acc_t = {}
            for v in range(NV):
                if v == 100:
                    # bias needed only at the kb NKB-2 fold; DMA it once the
                    # warmup bandwidth crunch is over
                    nc.gpsimd.dma_start(out=bias_tile[:], in_=bias_w[:, :])
                _, j, i = visits[v]
                kb, mb = M[j]
                kt = glob_kt(j, i)
                for fj, fq in fetch_at[v]:
                    emit_xt_pair(fj, fq)
                while build_cursor < KT and first_seen[build_cursor] < v + AHEAD:
                    emit_bdma(build_cursor)
                    emit_build(build_cursor)
                    build_cursor += 1
                if i == 0 and j not in ps_tiles:
                    ps_tiles[j] = psum_pool.tile([P, 4 * OC], mybir.dt.float32,
                                                 name="psw")
                ps = ps_tiles[j]
                xt = xt_tiles[(j, i // 2)]
                half = i % 2
                for jj in range(4):
                    nc.tensor.matmul(
                        ps[:, jj * OC:(jj + 1) * OC],
                        xt[:, half * 4 * P + jj * P:half * 4 * P + (jj + 1) * P],
                        w_tiles[kt][:],
                        start=(i == 0),
                        stop=(i == lens[j] - 1),
                    )
                if half == 1:
                    del xt_tiles[(j, i // 2)]
                # warmup fillers: pad the w1/w2 arrival chase with dummy
                # matmuls into mb1's not-yet-started psum tile (its slices
                # are reset by the real start=True matmuls), so the HAM
                # clock gate stays open through the chase
                if v < 2 and N_FILL:
                    if 1 not in ps_tiles:
                        ps_tiles[1] = psum_pool.tile(
                            [P, 4 * OC], mybir.dt.float32, name="psw")
                    for _ in range(N_FILL):
                        nc.tensor.matmul(ps_tiles[1][:, 0:OC], dummy_w[:],
                                         dummy_rhs[:], start=True, stop=True)
                # eviction: two half-wide DVE adds, hidden under the other
                # stream's solo visits
                if i == lens[j] - 1:
                    ps = ps_tiles.pop(j)
                    if kb == 0:
                        # psum -> acc as ACT-engine copies: keeps the DVE
                        # free for the warmup build chain. Bias is folded
                        # in later (kb NKB-2) when the DVE has slack.
                        acc = apool.tile([P, 4 * OC], mybir.dt.float32,
                                         name=f"acc_{mb}")
                        acc_t[mb] = acc
                        for h in range(2):
                            sl = slice(h * 2 * OC, (h + 1) * 2 * OC)
                            nc.scalar.copy(out=acc[:, sl], in_=ps[:, sl])
                    elif kb < NKB - 1:
                        acc = acc_t[mb]
                        for h in range(2):
                            sl = slice(h * 2 * OC, (h + 1) * 2 * OC)
                            nc.vector.tensor_tensor(
                                out=acc[:, sl], in0=ps[:, sl], in1=acc[:, sl],
                                op=mybir.AluOpType.add,
                            )
                        if kb == NKB - 2:
                            for h in range(2):
                                sl = slice(h * 2 * OC, (h + 1) * 2 * OC)
                                nc.vector.tensor_tensor(
                                    out=acc[:, sl], in0=bias_tile[:, sl],
                                    in1=acc[:, sl],
                                    op=mybir.AluOpType.add,
                                )
                    else:
                        o_tile = opool.tile([P, 4 * OC], mybir.dt.float32,
                                            name="o_wide")
                        for h in range(2):
                            sl = slice(h * 2 * OC, (h + 1) * 2 * OC)
                            nc.vector.tensor_tensor(
                                out=o_tile[:, sl], in0=ps[:, sl],
                                in1=acc_t[mb][:, sl],
                                op=mybir.AluOpType.add,
                            )
                            # stores alternate queues to stay under each
                            # queue's bandwidth share
                            oeng = nc.sync if mb % 2 == 0 else nc.scalar
                            for jj in (2 * h, 2 * h + 1):
                                m = mb * 4 + jj
                                oeng.dma_start(
                                    out=out[m * P:(m + 1) * P, :],
                                    in_=o_tile[:, jj * OC:(jj + 1) * OC],
                                )

    nc.compile()
    return nc


_NC_CACHE = None


def _get_program():
    global _NC_CACHE
    if _NC_CACHE is None:
        _NC_CACHE = _build_program()
    return _NC_CACHE


def prep_inputs(x, b, a, bias):
    """Host-side shard/cast/layout only. Returns per-core input maps."""
    x = np.asarray(x, dtype=np.float32)
    b = np.asarray(b, dtype=np.float32)
    a = np.asarray(a, dtype=np.float32)
    bias = np.asarray(bias, dtype=np.float32)
    xT16 = np.ascontiguousarray(x.T).astype(BF16)          # [I, B] bf16
    # pair-interleaved layout: xT4[ktp*128+p, mb*1024 + half*512 + c]
    #   == xT[(2*ktp+half)*128 + p, mb*512 + c]
    xT4 = np.ascontiguousarray(
        xT16.reshape(16, 2, 128, 8, 512).transpose(0, 2, 3, 1, 4)
    ).reshape(I // 2, 2 * B)
    b_iko = np.transpose(b, (1, 0, 2)).astype(BF16)        # [I, K, O] bf16
    bias32 = bias.astype(np.float32)
    a16 = a.astype(BF16)                                    # [K, O]

    in_maps = []
    for c in range(NCORES):
        sl = slice(c * OC, (c + 1) * OC)
        b_slice = np.ascontiguousarray(b_iko[:, :, sl]).reshape(I, K * OC)
        a_flat = np.ascontiguousarray(a16[:, sl]).reshape(1, K * OC)
        a_bcast = np.broadcast_to(a_flat, (P, K * OC)).copy()
        bias_wide = np.broadcast_to(
            np.tile(bias32[sl], 4).reshape(1, 4 * OC), (P, 4 * OC)).astype(BF16)
        in_maps.append({
            "b_re": b_slice,
            "a_b": a_bcast,
            "xT4": xT4,
            "bias_w": bias_wide,
        })
    return in_maps


def run(in_maps, trace=False):
    from concourse.bass_utils import run_bass_kernel_spmd

    nc = _get_program()
    res = run_bass_kernel_spmd(nc, in_maps, list(range(NCORES)), trace=trace)
    return res


def kernel(x, b, a, bias):
    in_maps = prep_inputs(x, b, a, bias)
    res = run(in_maps)
    out = np.concatenate([res.results[c]["out"] for c in range(NCORES)], axis=1)
    return np.ascontiguousarray(out, dtype=np.float32)


if __name__ == "__main__":
    rng = np.random.default_rng(0)
    x = rng.standard_normal((B, I), dtype=np.float32)
    b = rng.standard_normal((K, I, O), dtype=np.float32)
    a = rng.random((K, O), dtype=np.float32)
    bias = rng.standard_normal(O, dtype=np.float32)
    out = kernel(x=x, b=b, a=a, bias=bias)
    w_eff = np.einsum('kio,ko->io', np.sign(b), a.astype(np.float64)).astype(np.float64)
    expected = x.astype(np.float64) @ w_eff + bias
    rel = np.linalg.norm(out - expected) / np.linalg.norm(expected)
    print(f"rel_err = {rel:.3e}")


# revision 28
# speedup vs baseline: 1.0552x; 1.0552x over previous
"""Trainium2 Bass kernel for nn_BinaryDense: out = x @ (sum_k sign(b_k)*a_k) + bias.

Shapes (hardcoded): x [4096,4096] f32, b [4,4096,4096] f32, a [4,4096] f32,
bias [4096] f32 -> out [4096,4096] f32.

Strategy: tensor-parallel over the output (units) dim across 8 NeuronCores.
Core c owns O-columns [c*512, (c+1)*512).

Per core: one bf16 matmul x @ w with w built on-chip.
  w[:, oc] = sum_k copysign(a[k,oc], b[k,:,oc]); b arrives bf16 in
  [I, K, O_c] (k-major) layout. Build per 128-row k-tile is 3 DVE ops:
    contrib = (b & 0x80008000) | a   (one fused scalar_tensor_tensor, int32)
    t = contrib[0:2] + contrib[2:4]  (bf16 add, 1024 wide)
    w = t[0] + t[1]                  (bf16 add, 512 wide)

Schedule: software-pipelined mb stream. The 32 m-tiles form 8 mbs of 4 per
k-block; PSUM holds two wide [128, 2048] f32 tiles (4 banks each), i.e. two
mbs in flight. mb j+1's kt-sweep is offset ~half a sweep from mb j's, and
mb j+2 starts E extra visit-slots after mb j ends, so mb j's psum eviction
(two half-wide [128,1024] DVE adds into an fp32 SBUF accumulator) hides
under mb j+1's solo visits instead of stalling the PE. The pipeline flows
seamlessly across k-block boundaries. Per visit (mb, kt) the PE runs 4
matmuls (512 moving cols each); start/stop flags bound each k-block's
accumulation group per psum slice.

Measured hardware notes:
- Any GPSIMD Q7 activity (partition_broadcast, gpsimd tensor ops --
  anything needing LOAD_LIB) risks chip downclock; GpSimd issues DMA
  descriptors only. (The chip also lotteries between ~2.4 and ~2.0 GHz
  run-to-run regardless of the kernel.)
- DMA queue bandwidth divides ~equally among ACTIVE queues (~400-450
  GB/s aggregate, ~110-150 GB/s per busy queue during warmup), and the
  first HWDGE byte moves only at t~8us. So the warmup-critical tiles
  (a+b0..b3 and the first xt) are spread ACROSS the sync/scalar/gpsimd
  queues to land concurrently, bias is deferred to mid-kernel, and
  kb1's b tiles self-pace via bpool back-pressure.
- kb0's psum evictions are ACT-engine copies (bias folded in at kb
  NKB-2 when the DVE has slack) so the DVE warmup is builds-only.

DMA layout: xt comes as kt-PAIR tiles [128, 1024] (halves descriptor
count; each issue costs ~0.7us of queue time) from a host
pair-interleaved copy of x^T, alternating scalar/sync queues; out
stores alternate queues likewise.

Host side only reshapes/casts/shards (no math): x^T bf16 pair-interleaved,
b -> [I,K,O] bf16, a broadcast rows, bias tiled 4x broadcast rows (bf16).
"""

import sys

if "/opt/trn_rl_repo" not in sys.path:
    sys.path.insert(0, "/opt/trn_rl_repo")

import numpy as np
import ml_dtypes

BF16 = ml_dtypes.bfloat16

B = 4096   # batch rows of x
I = 4096   # input dim (contraction)
O = 4096   # output dim (sharded)
K = 4      # binary bases
NCORES = 8
OC = O // NCORES   # 512 output cols per core
P = 128

KT = I // P        # 32 k-tiles (contraction)
MT = B // P        # 32 m-tiles (output rows)
NMB = 8            # mbs (of 4 m-tiles) per k-block sweep

SIGNMASK = -2147450880  # 0x80008000: bf16 sign-bit pair as int32


def _build_program():
    import os
    import concourse.bass as bass
    import concourse.mybir as mybir
    from concourse import bacc
    from concourse.tile import TileContext

    nc = bacc.Bacc(None, target_bir_lowering=False)

    b_re = nc.declare_dram_parameter("b_re", [I, K * OC], mybir.dt.bfloat16, isOutput=False)
    a_b = nc.declare_dram_parameter("a_b", [P, K * OC], mybir.dt.bfloat16, isOutput=False)
    # x^T, pair-interleaved: [ktp*128+p, mb*1024 + half*512 + c]
    xT4 = nc.declare_dram_parameter("xT4", [I // 2, 2 * B], mybir.dt.bfloat16, isOutput=False)
    bias_w = nc.declare_dram_parameter("bias_w", [P, 4 * OC], mybir.dt.bfloat16, isOutput=False)
    out = nc.declare_dram_parameter("out", [B, OC], mybir.dt.float32, isOutput=True)

    K_BLOCKS = [int(s) for s in os.environ.get("BK_KBLOCKS", "4,4,6,8,10").split(",")]
    assert sum(K_BLOCKS) == KT
    assert all(kb % 2 == 0 for kb in K_BLOCKS)
    NKB = len(K_BLOCKS)
    k_starts = [sum(K_BLOCKS[:i]) for i in range(NKB)]
    N_DUM = int(os.environ.get("BK_DUMMIES", "20"))
    E_SLOTS = int(os.environ.get("BK_E", "3"))
    N_FILL = int(os.environ.get("BK_FILL", "0"))
    LOOKV = int(os.environ.get("BK_LOOK", "7"))
    AHEAD = int(os.environ.get("BK_AHEAD", "40"))

    # ---- software-pipeline schedule ----
    M = [(kb, mb) for kb in range(NKB) for mb in range(NMB)]
    lens = [K_BLOCKS[kb] for kb, mb in M]
    starts = []
    for j in range(len(M)):
        s = 0 if j == 0 else starts[j - 1] + (lens[j - 1] + 1) // 2
        if j >= 2:
            s = max(s, starts[j - 2] + lens[j - 2] + E_SLOTS)
        starts.append(s)
    for j in range(2, len(M)):
        assert starts[j] >= starts[j - 2] + lens[j - 2], "psum overcommit"
    visits = sorted(
        (starts[j] + i, j, i) for j in range(len(M)) for i in range(lens[j])
    )
    NV = len(visits)
    assert K_BLOCKS[0] >= 4 and NKB >= 3
    assert NV == NMB * KT  # 256 visits = 1024 matmuls / 4

    def glob_kt(j, i):
        return k_starts[M[j][0]] + i

    first_seen = {}
    vidx = {}
    for v, (_, j, i) in enumerate(visits):
        vidx[(j, i)] = v
        kt = glob_kt(j, i)
        if kt not in first_seen:
            first_seen[kt] = v
    assert sorted(first_seen) == list(range(KT))
    assert all(first_seen[k] <= first_seen[k + 1] for k in range(KT - 1))

    # xt pair-fetches: (j, q) covers visits (j, 2q) and (j, 2q+1).
    # Emit each fetch LOOKV visits before its first use.
    fetch_at = [[] for _ in range(NV)]
    n_fetch = 0
    for j in range(len(M)):
        for q in range(lens[j] // 2):
            use = vidx[(j, 2 * q)]
            fetch_at[max(0, use - LOOKV)].append((j, q))
            n_fetch += 1

    with TileContext(nc) as tc:
        with (
            tc.tile_pool(name="const", bufs=1) as const,
            tc.tile_pool(name="bpool", bufs=6) as bpool,
            tc.tile_pool(name="cpool", bufs=3) as cpool,
            tc.tile_pool(name="tpool", bufs=3) as tpool,
            tc.tile_pool(name="wpool", bufs=1) as wpool,
            tc.tile_pool(name="xpool", bufs=8) as xpool,
            tc.tile_pool(name="apool", bufs=1) as apool,
            tc.tile_pool(name="opool", bufs=2) as opool,
            tc.tile_pool(name="psum", bufs=2, space="PSUM") as psum_pool,
        ):
            # const tiles declared here; their DMAs are emitted inside the
            # hand-ordered prime sequence below (queue order = emission order)
            a_tile = const.tile([P, K * OC], mybir.dt.bfloat16)
            bias_tile = const.tile([P, 4 * OC], mybir.dt.bfloat16)

            mask_tile = const.tile([P, 1], mybir.dt.int32)
            nc.vector.memset(mask_tile[:], SIGNMASK)
            dummy_w = const.tile([P, P], mybir.dt.bfloat16)
            nc.vector.memset(dummy_w[:], 0)
            dummy_rhs = const.tile([P, OC], mybir.dt.bfloat16)
            nc.vector.memset(dummy_rhs[:], 0)
            act_warm = const.tile([P, 8], mybir.dt.float32)
            nc.scalar.copy(out=act_warm[:], in_=dummy_rhs.bitcast(mybir.dt.float32)[:, 0:8])

            # ---- w-build machinery ----
            b_live = {}
            w_tiles = [None] * KT

            def emit_bdma(kt):
                # queue bandwidth divides ~equally among ACTIVE queues, so
                # the warmup b tiles are spread across all three queues to
                # land concurrently; the rest rides sync (idle mid-kernel).
                b_tile = bpool.tile([P, K * OC], mybir.dt.bfloat16, name="b_tile")
                if kt == 0:
                    eng = nc.scalar
                elif kt == 1 or kt >= k_starts[2]:
                    eng = nc.sync
                else:
                    eng = nc.gpsimd
                eng.dma_start(out=b_tile[:], in_=b_re[kt * P:(kt + 1) * P, :])
                b_live[kt] = b_tile

            def emit_build(kt):
                b_tile = b_live.pop(kt)
                contrib = cpool.tile([P, K * OC], mybir.dt.bfloat16, name="contrib")
                nc.vector.scalar_tensor_tensor(
                    out=contrib.bitcast(mybir.dt.int32)[:],
                    in0=b_tile.bitcast(mybir.dt.int32)[:],
                    scalar=mask_tile[:, 0:1],
                    in1=a_tile.bitcast(mybir.dt.int32)[:],
                    op0=mybir.AluOpType.bitwise_and,
                    op1=mybir.AluOpType.bitwise_or,
                )
                t_tile = tpool.tile([P, 2 * OC], mybir.dt.bfloat16, name="t_tile")
                nc.vector.tensor_tensor(
                    out=t_tile[:],
                    in0=contrib[:, 0:2 * OC],
                    in1=contrib[:, 2 * OC:4 * OC],
                    op=mybir.AluOpType.add,
                )
                w_tile = wpool.tile([P, OC], mybir.dt.bfloat16, name=f"w_{kt}")
                nc.vector.tensor_tensor(
                    out=w_tile[:],
                    in0=t_tile[:, 0:OC],
                    in1=t_tile[:, OC:2 * OC],
                    op=mybir.AluOpType.add,
                )
                w_tiles[kt] = w_tile

            xt_tiles = {}

            def emit_xt_pair(j, q):
                if (j, q) in xt_tiles:
                    return
                kb, mb = M[j]
                ktp = (k_starts[kb] + 2 * q) // 2
                xt = xpool.tile([P, 8 * P], mybir.dt.bfloat16, name="xt")
                xeng = nc.scalar if (j + q) % 2 == 0 else nc.sync
                xeng.dma_start(
                    out=xt[:],
                    in_=xT4[ktp * P:(ktp + 1) * P, mb * 8 * P:(mb + 1) * 8 * P],
                )
                xt_tiles[(j, q)] = xt

            # ---- prime: hand-ordered critical chain on the sync queue:
            # b0, a, X00, b1, X01, b2, X10, b3, X11, X20. Bias (bf16) heads
            # the gpsimd queue, followed later by kb1's b tiles. ----
            n_prime = 0
            while n_prime < KT and first_seen[n_prime] < AHEAD:
                n_prime += 1
            nc.sync.dma_start(out=a_tile[:], in_=a_b[:, :])
            emit_bdma(0)          # scalar head
            emit_bdma(1)          # sync (behind a)
            emit_bdma(3)          # gpsimd head
            emit_bdma(2)          # gpsimd
            emit_xt_pair(0, 0)    # scalar (behind b0)
            emit_build(0)
            emit_build(1)
            emit_xt_pair(0, 1)
            emit_build(2)
            emit_xt_pair(1, 0)
            emit_build(3)
            emit_xt_pair(1, 1)
            emit_xt_pair(2, 0)
            build_cursor = 4
            while build_cursor < n_prime:
                emit_bdma(build_cursor)
                emit_build(build_cursor)
                build_cursor += 1

            for _ in range(N_DUM):
                dps = psum_pool.tile([P, 4 * OC], mybir.dt.float32, name="psw")
                nc.tensor.matmul(dps[:, 0:OC], dummy_w[:], dummy_rhs[:],
                                 start=True, stop=True)

            # ---- main loop over scheduled visits ----
            ps_tiles = {}
            acc_t = {}
            for v in range(NV):
                if v == 100:
                    # bias needed only at the kb NKB-2 fold; DMA it once the
                    # warmup bandwidth crunch is over
                    nc.gpsimd.dma_start(out=bias_tile[:], in_=bias_w[:, :])
                _, j, i = visits[v]
                kb, mb = M[j]
                kt = glob_kt(j, i)
                for fj, fq in fetch_at[v]:
                    emit_xt_pair(fj, fq)
                while build_cursor < KT and first_seen[build_cursor] < v + AHEAD:
                    emit_bdma(build_cursor)
                    emit_build(build_cursor)
                    build_cursor += 1
                if i == 0 and j not in ps_tiles:
                    ps_tiles[j] = psum_pool.tile([P, 4 * OC], mybir.dt.float32,
                                                 name="psw")
                ps = ps_tiles[j]
                xt = xt_tiles[(j, i // 2)]
                half = i % 2
                for jj in range(4):
                    nc.tensor.matmul(
                        ps[:, jj * OC:(jj + 1) * OC],
                        xt[:, half * 4 * P + jj * P:half * 4 * P + (jj + 1) * P],
                        w_tiles[kt][:],
                        start=(i == 0),
                        stop=(i == lens[j] - 1),
                    )
                if half == 1:
                    del xt_tiles[(j, i // 2)]
                # warmup fillers (off by default: a denser early matmul
                # stream was measured to trip a power-throttle HAM state)
                if v < 2 and N_FILL:
                    if 1 not in ps_tiles:
                        ps_tiles[1] = psum_pool.tile(
                            [P, 4 * OC], mybir.dt.float32, name="psw")
                    for _ in range(N_FILL):
                        nc.tensor.matmul(ps_tiles[1][:, 0:OC], dummy_w[:],
                                         dummy_rhs[:], start=True, stop=True)
                # eviction: two half-wide DVE adds, hidden under the other
                # stream's solo visits
                if i == lens[j] - 1:
                    ps = ps_tiles.pop(j)
                    if kb == 0:
                        # psum -> acc as ACT-engine copies: keeps the DVE
                        # free for the warmup build chain. Bias is folded
                        # in later (kb NKB-2) when the DVE has slack.
                        acc = apool.tile([P, 4 * OC], mybir.dt.float32,
                                         name=f"acc_{mb}")
                        acc_t[mb] = acc
                        for h in range(2):
                            sl = slice(h * 2 * OC, (h + 1) * 2 * OC)
                            nc.scalar.copy(out=acc[:, sl], in_=ps[:, sl])
                    elif kb < NKB - 1:
                        acc = acc_t[mb]
                        for h in range(2):
                            sl = slice(h * 2 * OC, (h + 1) * 2 * OC)
                            nc.vector.tensor_tensor(
                                out=acc[:, sl], in0=ps[:, sl], in1=acc[:, sl],
                                op=mybir.AluOpType.add,
                            )
                        if kb == NKB - 2:
                            for h in range(2):
                                sl = slice(h * 2 * OC, (h + 1) * 2 * OC)
                                nc.vector.tensor_tensor(
                                    out=acc[:, sl], in0=bias_tile[:, sl],
                                    in1=acc[:, sl],
                                    op=mybir.AluOpType.add,
                                )
                    else:
                        o_tile = opool.tile([P, 4 * OC], mybir.dt.float32,
                                            name="o_wide")
                        for h in range(2):
                            sl = slice(h * 2 * OC, (h + 1) * 2 * OC)
                            nc.vector.tensor_tensor(
                                out=o_tile[:, sl], in0=ps[:, sl],
                                in1=acc_t[mb][:, sl],
                                op=mybir.AluOpType.add,
                            )
                            # stores alternate queues to stay under each
                            # queue's bandwidth share
                            oeng = nc.sync if mb % 2 == 0 else nc.scalar
                            for jj in (2 * h, 2 * h + 1):
                                m = mb * 4 + jj
                                oeng.dma_start(
                                    out=out[m * P:(m + 1) * P, :],
                                    in_=o_tile[:, jj * OC:(jj + 1) * OC],
                                )

    nc.compile()
    return nc


_NC_CACHE = None


def _get_program():
    global _NC_CACHE
    if _NC_CACHE is None:
        _NC_CACHE = _build_program()
    return _NC_CACHE


def prep_inputs(x, b, a, bias):
    """Host-side shard/cast/layout only. Returns per-core input maps."""
    x = np.asarray(x, dtype=np.float32)
    b = np.asarray(b, dtype=np.float32)
    a = np.asarray(a, dtype=np.float32)
    bias = np.asarray(bias, dtype=np.float32)
    xT16 = np.ascontiguousarray(x.T).astype(BF16)          # [I, B] bf16
    # pair-interleaved layout: xT4[ktp*128+p, mb*1024 + half*512 + c]
    #   == xT[(2*ktp+half)*128 + p, mb*512 + c]
    xT4 = np.ascontiguousarray(
        xT16.reshape(16, 2, 128, 8, 512).transpose(0, 2, 3, 1, 4)
    ).reshape(I // 2, 2 * B)
    b_iko = np.transpose(b, (1, 0, 2)).astype(BF16)        # [I, K, O] bf16
    bias32 = bias.astype(np.float32)
    a16 = a.astype(BF16)                                    # [K, O]

    in_maps = []
    for c in range(NCORES):
        sl = slice(c * OC, (c + 1) * OC)
        b_slice = np.ascontiguousarray(b_iko[:, :, sl]).reshape(I, K * OC)
        a_flat = np.ascontiguousarray(a16[:, sl]).reshape(1, K * OC)
        a_bcast = np.broadcast_to(a_flat, (P, K * OC)).copy()
        bias_wide = np.broadcast_to(
            np.tile(bias32[sl], 4).reshape(1, 4 * OC), (P, 4 * OC)).astype(BF16)
        in_maps.append({
            "b_re": b_slice,
            "a_b": a_bcast,
            "xT4": xT4,
            "bias_w": bias_wide,
        })
    return in_maps


def run(in_maps, trace=False):
    from concourse.bass_utils import run_bass_kernel_spmd

    nc = _get_program()
    res = run_bass_kernel_spmd(nc, in_maps, list(range(NCORES)), trace=trace)
    return res


def kernel(x, b, a, bias):
    in_maps = prep_inputs(x, b, a, bias)
    res = run(in_maps)
    out = np.concatenate([res.results[c]["out"] for c in range(NCORES)], axis=1)
    return np.ascontiguousarray(out, dtype=np.float32)


if __name__ == "__main__":
    rng = np.random.default_rng(0)
    x = rng.standard_normal((B, I), dtype=np.float32)
    b = rng.standard_normal((K, I, O), dtype=np.float32)
    a = rng.random((K, O), dtype=np.float32)
    bias = rng.standard_normal(O, dtype=np.float32)
    out = kernel(x=x, b=b, a=a, bias=bias)
    w_eff = np.einsum('kio,ko->io', np.sign(b), a.astype(np.float64)).astype(np.float64)
    expected = x.astype(np.float64) @ w_eff + bias
    rel = np.linalg.norm(out - expected) / np.linalg.norm(expected)
    print(f"rel_err = {rel:.3e}")
